# revision 51
# baseline (speedup 1.0000x reference)
"""MultiHeadLatentAttention TRN2 kernel (two-program, latent-TP).

Sharding: 8 cores = 2 (batch) x 4 (head groups of 4 heads).  Two device
programs with a host-side latent relay between them:

Program 1 (down): the shared latent down-projections (Wkv_d, Wq_d) are
tensor-parallel split across the 4 cores of each batch group - each core
computes a distinct 128-row shard of kv_d AND q_d over all S (16-deep
E-contraction, x resident as contiguous [128, S] e-chunk tiles).  The
host concatenates the shards into full [2*LAT, S] latents per batch
(pure layout, no host compute) - this is what makes MLA's low-rank
structure pay: every core then contracts over LAT=512 instead of E=2048
for the k1/q1/qr/v up-projections, cutting per-core matmul count ~30%
versus folding W_up @ W_down into direct E->head projections.

Program 2 (main): per core, for its batch and its 4 heads: up-project
K^T/Q^T (with RoPE; rope-k comes straight from x) and V from the
latents, full attention, and a partial output projection (its 512 rows
of Wo's input dim).  bf16 partial outputs are summed on the host in
fp32 (+ bo).  Down-projection biases are folded into the up biases.

All matmul operands bf16 (fp32 PSUM accumulation).  Schedule is built
around the PE array (the global bottleneck), keeping the ACT exp stream
and DVE row-accum off the critical path:

  A: per s-chunk: k1/V from latents, rope-k from x; q-side for s-chunks
     0,1 inline.  PSUM bias-consumers run on ACT (idle here) so the
     2-slot pac PSUM rotation never waits on the DVE stream.
  B half 0: 4 head streams; q-side projections for s-chunks 2,3 are
     interleaved between streams as exp-independent PE filler.
  B half 1: 4 head streams with all 8 C-half-0 output blocks
     interleaved (PSUM drains alternate DVE/ACT); C half 1 at the end
     overlaps the next iteration's DMA/DVE leads in the For_i loop.

  B stream: QK (2 MM) -> exp on ACT -> bf16 row-accum on DVE -> PV
  trailing by LAG chunks; PV accumulators drain to bf16 att tiles
  (overwriting consumed Q columns) on DVE.  Softmax tail (ones-matmul
  row sums on PE, reciprocal+normalize on DVE, partition-broadcast on
  Pool) is software-pipelined 1-2 streams behind.  Softmax skips
  max-subtraction (scores are bounded).
"""
import sys

sys.path.insert(0, "/opt/trn_rl_repo")

from contextlib import ExitStack

import numpy as np

H = 16
E = 2048
LAT = E // 4          # 512
D = E // H            # 128
R = D // 2            # 64
B, S = 2, 2048
HPC = H // 4          # 4 heads per core
NCORES = 8
NE = E // 128         # 16 contraction chunks over E
NL = LAT // 128       # 4 contraction chunks over LAT
SW = 512              # s-chunk width for projections
NSC = S // SW         # 4 s-chunks
NKC = S // 128        # 16 key chunks
SCALE = 1.0 / float(np.sqrt(D))
LAG = 5               # PV trails QK/exp by LAG k-chunks

_RT = {}  # cached runtimes
_LAT_CACHE = {}  # batch -> gathered latents (bf16 [2*LAT, S]), set by kernel()


def _mk_down(nc):
    """Down-program DRAM I/O: x, the core's latent-shard weights, lat out.

    Each core computes a distinct 128-row shard of kv_d AND of q_d for
    its batch, over all S (tensor-parallel split of the shared latent
    down-projections across the 4 cores of a batch group; the host
    gathers the shards between the two programs)."""
    import concourse.mybir as mybir
    BF16 = mybir.dt.bfloat16
    d = {}
    d["xT"] = nc.dram_tensor("xT", [E, S], BF16, kind="ExternalInput")
    # [E, 256]: cols 0:128 = kv_d shard rows, 128:256 = q_d shard rows
    d["wdn"] = nc.dram_tensor("wdn", [E, 256], BF16, kind="ExternalInput")
    d["lat"] = nc.dram_tensor("lat", [256, S], BF16, kind="ExternalOutput")
    return d


def _emit_down_body(nc, tc, d, wdn, p):
    """Two double-s-chunk passes: each loads a [128, NE*1024] x gather
    (2 KB DMA lines, full HBM rate) consumed only by its own pass's four
    projections - the tile retires mid-iteration, so the next
    iteration's x traffic overlaps this iteration's matmuls.  Each
    projection: 16-deep E-contraction, ACT-drained to bf16, DMA'd out."""
    import concourse.mybir as mybir
    F32 = mybir.dt.float32
    BF16 = mybir.dt.bfloat16
    W2 = 2 * SW
    for half in range(2):
        xt = p["xa"].tile([128, NE * W2], BF16, name="xt2")
        nc.sync.dma_start(
            xt[:].rearrange("p (e s) -> p e s", e=NE),
            d["xT"][:, half * W2:(half + 1) * W2]
            .rearrange("(e p) s -> p e s", p=128))
        for sc2 in range(2):
            sc = half * 2 + sc2
            ssl = slice(sc * SW, (sc + 1) * SW)
            for m in range(2):
                ps = p["pac"].tile([128, SW], F32, name="psD", tag="pac")
                for e in range(NE):
                    nc.tensor.matmul(
                        ps[:],
                        wdn[:, e * 256 + m * 128:e * 256 + (m + 1) * 128],
                        xt[:, e * W2 + sc2 * SW:e * W2 + (sc2 + 1) * SW],
                        start=(e == 0), stop=(e == NE - 1))
                ob = p["oc"].tile([128, SW], BF16, name="obd")
                nc.scalar.copy(ob[:], ps[:])
                nc.sync.dma_start(d["lat"][m * 128:(m + 1) * 128, ssl],
                                  ob[:])


def _build_down(loop_n=None):
    """The down-projection program (prog 1).  loop_n: For_i repeat count
    for benchmarking (None = single pass)."""
    import concourse.bacc as bacc
    import concourse.mybir as mybir
    import concourse.tile as tile

    BF16 = mybir.dt.bfloat16
    nc = bacc.Bacc("TRN2", target_bir_lowering=False, debug=False,
                   num_devices=NCORES)
    d = _mk_down(nc)
    with tile.TileContext(nc) as tc, ExitStack() as top:
        wpool = top.enter_context(tc.tile_pool(name="wp", bufs=1))
        wdn = wpool.tile([128, NE * 256], BF16, name="wdn_t")
        nc.sync.dma_start(
            wdn[:].rearrange("p (e c) -> p e c", e=NE),
            d["wdn"][:].rearrange("(e p) c -> p e c", p=128))
        p = {}
        p["xa"] = top.enter_context(tc.tile_pool(name="xa", bufs=2))
        p["oc"] = top.enter_context(tc.tile_pool(name="oc", bufs=4))
        p["pac"] = top.enter_context(tc.tile_pool(name="pac", bufs=2,
                                                  space="PSUM"))
        if loop_n is None or loop_n == 1:
            _emit_down_body(nc, tc, d, wdn, p)
        else:
            with tc.For_i(0, loop_n, 1):
                _emit_down_body(nc, tc, d, wdn, p)
    nc.compile()
    return nc


def _mk(nc):
    """Declare DRAM I/O; returns dict of handles."""
    import concourse.mybir as mybir
    F32 = mybir.dt.float32
    BF16 = mybir.dt.bfloat16
    d = {}
    d["xT"] = nc.dram_tensor("xT", [E, S], BF16, kind="ExternalInput")
    d["latT"] = nc.dram_tensor("latT", [2 * LAT, S], BF16,
                               kind="ExternalInput")
    for nm in ("wk1u", "wq1u", "wqru"):
        d[nm] = nc.dram_tensor(nm, [LAT, HPC * R], BF16,
                               kind="ExternalInput")
    d["wvu"] = nc.dram_tensor("wvu", [LAT, HPC * D], BF16,
                              kind="ExternalInput")
    d["wrkT"] = nc.dram_tensor("wrkT", [E, HPC * R], BF16,
                               kind="ExternalInput")
    d["woT"] = nc.dram_tensor("woT", [HPC * D, E], BF16,
                              kind="ExternalInput")
    for nm in ("bk1f", "bq1f", "bqrf", "brk"):
        d[nm] = nc.dram_tensor(nm, [128, 2], F32, kind="ExternalInput")
    d["bvf"] = nc.dram_tensor("bvf", [1, HPC * D], F32,
                              kind="ExternalInput")
    d["onesd"] = nc.dram_tensor("onesd", [128, 1], BF16,
                                kind="ExternalInput")
    d["cosT"] = nc.dram_tensor("cosT", [128, S], BF16,
                               kind="ExternalInput")
    d["sinT"] = nc.dram_tensor("sinT", [128, S], BF16,
                               kind="ExternalInput")
    d["out"] = nc.dram_tensor("out", [S, E], BF16, kind="ExternalOutput")
    return d


def _consts(nc, tc, top, d):
    """Persistent tiles: K/Q/V storage, biases, ones, weights, cos/sin."""
    import concourse.mybir as mybir
    F32 = mybir.dt.float32
    BF16 = mybir.dt.bfloat16

    kq_pool = top.enter_context(tc.tile_pool(name="kq", bufs=1))
    v_pool = top.enter_context(tc.tile_pool(name="vp", bufs=1))
    cpool = top.enter_context(tc.tile_pool(name="cp", bufs=1))
    wpool = top.enter_context(tc.tile_pool(name="wp", bufs=1))

    t = {}
    t["K"] = [kq_pool.tile([128, S], BF16, name=f"Kt{h}") for h in range(HPC)]
    t["Q"] = [kq_pool.tile([128, S], BF16, name=f"Qt{h}") for h in range(HPC)]
    t["V"] = [v_pool.tile([128, HPC * D], BF16, name=f"Vt{i}")
              for i in range(NKC)]

    def ld(name, dram, shape, dt=F32):
        tl = cpool.tile(shape, dt, name=name)
        nc.sync.dma_start(tl[:], dram[:])
        return tl

    t["ones"] = ld("ones_t", d["onesd"], [128, 1], BF16)
    t["bk1f"] = ld("bk1f_t", d["bk1f"], [128, 2])
    t["bq1f"] = ld("bq1f_t", d["bq1f"], [128, 2])
    t["bqrf"] = ld("bqrf_t", d["bqrf"], [128, 2])
    t["brk"] = ld("brk_t", d["brk"], [128, 2])
    t["cos"] = ld("cos_t", d["cosT"], [128, S], BF16)
    t["sin"] = ld("sin_t", d["sinT"], [128, S], BF16)
    bvf_row = ld("bvf_row", d["bvf"], [1, HPC * D])
    bvf_bc = cpool.tile([128, HPC * D], F32, name="bvf_bc")
    nc.gpsimd.partition_broadcast(bvf_bc[:], bvf_row[:])
    t["bvf_bc"] = bvf_bc

    # projection weights, loaded once, ordered by first use so the first
    # A pass isn't blocked behind later weights on the DMA queue.
    # Latent up-projections fold contraction as (l p) -> p l (NL=4
    # chunks); rk keeps the E-deep fold (NE=16 chunks).
    dnw = {}
    for nm, key, nch, cw in (("k1u", "wk1u", NL, HPC * R),
                             ("rk", "wrkT", NE, HPC * R),
                             ("vu", "wvu", NL, HPC * D),
                             ("q1u", "wq1u", NL, HPC * R),
                             ("qru", "wqru", NL, HPC * R)):
        tl = wpool.tile([128, nch * cw], BF16, name=f"wd{nm}")
        nc.sync.dma_start(
            tl[:].rearrange("p (e c) -> p e c", e=nch),
            d[key][:].rearrange("(e p) c -> p e c", p=128))
        dnw[nm] = (tl, cw)
    t["dnw"] = dnw

    # output projection weights
    t["wo"] = [wpool.tile([128, E], BF16, name=f"wo{hc}")
               for hc in range(HPC)]
    for hc in range(HPC):
        nc.sync.dma_start(t["wo"][hc][:], d["woT"][hc * 128:(hc + 1) * 128, :])
    return t


def _pools(nc, tc, st):
    import concourse.mybir as mybir  # noqa: F401
    p = {}
    p["xa"] = st.enter_context(tc.tile_pool(name="xa", bufs=1))
    p["lat"] = st.enter_context(tc.tile_pool(name="lat", bufs=1))
    p["rp"] = st.enter_context(tc.tile_pool(name="rp", bufs=1))
    p["pe"] = st.enter_context(tc.tile_pool(name="pe", bufs=3))
    p["cb"] = st.enter_context(tc.tile_pool(name="cb", bufs=2))
    p["oc"] = st.enter_context(tc.tile_pool(name="oc", bufs=2))
    # PSUM: pac 2x[128,512] (A chains, C groups, row-sum outputs),
    # psS 2x[128,1024] (score tiles), psO 2x[128,512] (PV accumulators)
    p["pac"] = st.enter_context(tc.tile_pool(name="pac", bufs=2,
                                             space="PSUM"))
    p["psS"] = st.enter_context(tc.tile_pool(name="psS", bufs=2,
                                             space="PSUM"))
    p["psO"] = st.enter_context(tc.tile_pool(name="psO", bufs=1,
                                             space="PSUM"))
    return p


def _proj(nc, t, p, wname, m, xt, xw=SW, xoff=0):
    """x @ W chunk: 16-deep contraction over E, [128, SW] psum out.
    xt holds [128, NE*xw] with this pass's columns at xoff."""
    import concourse.mybir as mybir
    F32 = mybir.dt.float32
    wt, cw = t["dnw"][wname]
    ps = p["pac"].tile([128, SW], F32, name="psA", tag="pac")
    for e in range(NE):
        nc.tensor.matmul(
            ps[:], wt[:, e * cw + m * 128:e * cw + (m + 1) * 128],
            xt[:, e * xw + xoff:e * xw + xoff + SW],
            start=(e == 0), stop=(e == NE - 1))
    return ps


def _proj_lat(nc, t, p, wname, m, lats, ssl, dst=None):
    """latent @ W_up chunk: 4-deep contraction over LAT, [128, SW] out.

    dst=(tile, col) accumulates into a half of a wider PSUM tile (the
    psS score tiles are idle during the A section - using their 4 banks
    as extra rotation slots keeps the PE from stalling on the 2-slot pac
    pool's consumer latency)."""
    import concourse.mybir as mybir
    F32 = mybir.dt.float32
    wt, cw = t["dnw"][wname]
    if dst is None:
        ps = p["pac"].tile([128, SW], F32, name="psA", tag="pac")
        out = ps[:]
    else:
        ps, col = dst
        out = ps[:, col:col + SW]
    for l in range(NL):
        nc.tensor.matmul(
            out, wt[:, l * cw + m * 128:l * cw + (m + 1) * 128],
            lats[l][:, ssl],
            start=(l == 0), stop=(l == NL - 1))
    return None if dst is not None else ps


def _rope(nc, t, p, ps_ap, bias_t, m, dst, ssl, rope_on_act=True):
    """RoPE rows: ps [128 = 2 heads x 64 rope rows, SW] -> dst rows R:D.

    The PSUM consumer (bias add) runs on ACT (idle during projections)
    so the pac slot frees without waiting on the DVE stream; in B half 0
    (ACT busy with exps) it goes to DVE instead."""
    import concourse.mybir as mybir
    from concourse.alu_op_type import AluOpType
    BF16 = mybir.dt.bfloat16
    AF = mybir.ActivationFunctionType
    swap_mask = [i ^ 1 for i in range(32)]
    cos_s, sin_s = t["cos"][:, ssl], t["sin"][:, ssl]
    xb = p["rp"].tile([128, SW], BF16, name="xb")
    if rope_on_act:
        nc.scalar.activation(xb[:], ps_ap, AF.Identity,
                             bias=bias_t[:, m:m + 1])
    else:
        nc.vector.tensor_scalar_add(xb[:], ps_ap, bias_t[:, m:m + 1])
    sh = p["rp"].tile([128, SW], BF16, name="sh")
    nc.vector.stream_shuffle(sh[:], xb[:], swap_mask)
    t1 = p["rp"].tile([128, SW], BF16, name="t1")
    nc.vector.tensor_tensor(t1[:], xb[:], cos_s, op=AluOpType.mult)
    t2 = p["rp"].tile([128, SW], BF16, name="t2")
    nc.vector.tensor_tensor(t2[:], sh[:], sin_s, op=AluOpType.mult)
    nc.vector.tensor_tensor(dst[2 * m][R:D, ssl], t1[0:R, :],
                            t2[0:R, :], op=AluOpType.add)
    nc.vector.tensor_tensor(dst[2 * m + 1][R:D, ssl], t1[R:D, :],
                            t2[R:D, :], op=AluOpType.add)


def _emit_q_side(nc, t, p, sc, on_act=True):
    """q1 + rope-q projections for s-chunk sc from the q latents.

    on_act=False routes the PSUM bias-consumers to DVE instead - used
    for the q-sides interleaved into B half 0, where ACT is the pacing
    engine (the exp stream) but DVE has slack."""
    ssl = slice(sc * SW, (sc + 1) * SW)
    Q_t = t["Q"]
    lq = t["lat_q"]
    import concourse.mybir as mybir
    AF = mybir.ActivationFunctionType
    for m in range(2):
        ps = _proj_lat(nc, t, p, "q1u", m, lq, ssl)
        nc.scalar.activation(Q_t[2 * m][0:R, ssl], ps[0:R, :],
                             AF.Identity, bias=t["bq1f"][0:R, m:m + 1])
        nc.scalar.activation(Q_t[2 * m + 1][0:R, ssl], ps[R:D, :],
                             AF.Identity, bias=t["bq1f"][R:D, m:m + 1])
        ps = _proj_lat(nc, t, p, "qru", m, lq, ssl)
        _rope(nc, t, p, ps[:], t["bqrf"], m, Q_t, ssl)


def _emit_A_pass(nc, tc, d, t, p, sc, do_q, xt2=None):
    """One merged pass for s-chunk sc: k1/V from the kv latents, rope-k
    from x (+ q-side from the q latents if do_q).  x arrives in
    double-s-chunk gather tiles (2 KB DMA lines): loaded on even sc,
    reused on odd sc."""
    import concourse.mybir as mybir
    from concourse.alu_op_type import AluOpType
    F32 = mybir.dt.float32
    BF16 = mybir.dt.bfloat16
    K_t, V_t = t["K"], t["V"]
    lkv = t["lat_kv"]
    ssl = slice(sc * SW, (sc + 1) * SW)
    W2 = 2 * SW

    if xt2 is None:
        half = sc // 2
        xt2 = p["xa"].tile([128, NE * W2], BF16, name="xt2")
        nc.sync.dma_start(
            xt2[:].rearrange("p (e s) -> p e s", e=NE),
            d["xT"][:, half * W2:(half + 1) * W2]
            .rearrange("(e p) s -> p e s", p=128))
    xoff = (sc % 2) * SW

    AF = mybir.ActivationFunctionType
    for m in range(2):  # k1 -> K rows 0..63
        ps = _proj_lat(nc, t, p, "k1u", m, lkv, ssl)
        nc.scalar.activation(K_t[2 * m][0:R, ssl], ps[0:R, :], AF.Identity,
                             bias=t["bk1f"][0:R, m:m + 1])
        nc.scalar.activation(K_t[2 * m + 1][0:R, ssl], ps[R:D, :],
                             AF.Identity, bias=t["bk1f"][R:D, m:m + 1])
    for m in range(2):  # rope-k from x
        ps = _proj(nc, t, p, "rk", m, xt2, xw=W2, xoff=xoff)
        _rope(nc, t, p, ps[:], t["brk"], m, K_t, ssl)
    for j in range(SW // 128):  # V (s, feat) layout, 4-deep over LAT
        wt, cw = t["dnw"]["vu"]
        ps = p["pac"].tile([128, HPC * D], F32, name="psA", tag="pac")
        for l in range(NL):
            nc.tensor.matmul(
                ps[:], lkv[l][:, sc * SW + j * 128:sc * SW + (j + 1) * 128],
                wt[:, l * cw:(l + 1) * cw],
                start=(l == 0), stop=(l == NL - 1))
        nc.vector.tensor_tensor(V_t[sc * (SW // 128) + j][:], ps[:],
                                t["bvf_bc"][:], op=AluOpType.add)
    if do_q:
        _emit_q_side(nc, t, p, sc)
    return xt2


def _emit_B_half(nc, tc, d, t, p, att_t, qp, mode="full", between=None):
    import concourse.mybir as mybir
    from concourse.alu_op_type import AluOpType
    F32 = mybir.dt.float32
    BF16 = mybir.dt.bfloat16
    AF = mybir.ActivationFunctionType
    K_t, Q_t, V_t = t["K"], t["Q"], t["V"]

    def stream(h):
        """QK/exp/row-acc/PV for head h; returns deferred-tail closures.

        The softmax tail (row-sum matmuls, 1/r, broadcast, normalize) is
        deferred 1-2 streams so its small ops land BEHIND the next
        stream's work in each engine's in-order queue.  PV accumulators
        drain to SBUF (unnormalized bf16 att tiles) on Pool as soon as
        PV finishes, freeing the PSUM slots.
        """
        qa = slice(qp * 1024, qp * 1024 + 512)
        qb = slice(qp * 1024 + 512, (qp + 1) * 1024)
        oA = p["psO"].tile([128, 512], F32, name="oA")
        oB = p["psO"].tile([128, 512], F32, name="oB")
        acc0 = p["pe"].tile([128, 1024], BF16, name="acc0", bufs=2)
        acc1 = p["pe"].tile([128, 1024], BF16, name="acc1", bufs=2)
        accs = (acc0, acc1)
        pes = {}

        def pv(kk):
            pet = pes.pop(kk)
            nc.tensor.matmul(oA[:], V_t[kk][:, h * D:(h + 1) * D],
                             pet[:, 0:512], start=(kk == 0),
                             stop=(kk == NKC - 1))
            nc.tensor.matmul(oB[:], V_t[kk][:, h * D:(h + 1) * D],
                             pet[:, 512:1024], start=(kk == 0),
                             stop=(kk == NKC - 1))

        for kk in range(NKC):
            ksl = slice(kk * 128, (kk + 1) * 128)
            pp = p["psS"].tile([128, 1024], F32, name="pp")
            nc.tensor.matmul(pp[:, 0:512], K_t[h][:, ksl], Q_t[h][:, qa],
                             start=True, stop=True)
            nc.tensor.matmul(pp[:, 512:1024], K_t[h][:, ksl],
                             Q_t[h][:, qb], start=True, stop=True)
            if mode == "qk":
                continue
            pet = p["pe"].tile([128, 1024], BF16, name="pet", bufs=8)
            nc.scalar.activation(pet[:], pp[:], AF.Exp, scale=SCALE)
            if mode == "qke":
                continue
            if mode != "qkep":
                acc = accs[kk % 2]
                if kk < 2:
                    nc.vector.tensor_copy(acc[:], pet[:])
                else:
                    nc.vector.tensor_tensor(acc[:], pet[:], acc[:],
                                            op=AluOpType.add)
            if mode != "qkea":
                pes[kk] = pet
                if kk >= LAG:
                    pv(kk - LAG)
        if mode != "full":
            return None
        for kk in range(NKC - LAG, NKC):
            pv(kk)
        # drain PV accumulators (f32 PSUM -> bf16 att SBUF) on DVE
        nc.vector.tensor_copy(att_t[h][:, qa], oA[:])
        nc.vector.tensor_copy(att_t[h][:, qb], oB[:])
        nc.vector.tensor_tensor(acc0[:], acc1[:], acc0[:],
                                op=AluOpType.add)

        def tail1():
            sumA = p["pac"].tile([1, 512], F32, name="sumA", tag="pac")
            nc.tensor.matmul(sumA[:], t["ones"][:], acc0[:, 0:512],
                             start=True, stop=True)
            sumB = p["pac"].tile([1, 512], F32, name="sumB", tag="pac")
            nc.tensor.matmul(sumB[:], t["ones"][:], acc0[:, 512:1024],
                             start=True, stop=True)
            ci = p["cb"].tile([1, 1024], BF16, name="ci")
            with nc.allow_low_precision("softmax denom recip in bf16"):
                nc.vector.reciprocal(ci[:, 0:512], sumA[:])
                nc.vector.reciprocal(ci[:, 512:1024], sumB[:])
            return ci

        def tail2(ci):
            bcT = p["cb"].tile([128, 1024], BF16, name="bcT")
            nc.gpsimd.partition_broadcast(bcT[:], ci[:])
            nc.vector.tensor_tensor(att_t[h][:, qa], att_t[h][:, qa],
                                    bcT[:, 0:512], op=AluOpType.mult)
            nc.vector.tensor_tensor(att_t[h][:, qb], att_t[h][:, qb],
                                    bcT[:, 512:1024], op=AluOpType.mult)

        return tail1, tail2

    # 3-stage software pipeline: stream h's row-sum+recip lands behind
    # stream h+1; its broadcast+normalize behind stream h+2.
    from collections import deque
    pend = deque()  # [tail1, tail2, ci]
    for h in range(HPC):
        if between is not None:
            between(h)
        pair = stream(h)
        if pair is None:
            continue
        pend.append(list(pair) + [None])
        if len(pend) >= 2:
            e = pend[-2]
            e[2] = e[0]()
        if len(pend) >= 3:
            e = pend.popleft()
            if e[2] is None:
                e[2] = e[0]()
            e[1](e[2])
    while pend:
        e = pend.popleft()
        if e[2] is None:
            e[2] = e[0]()
        e[1](e[2])


def _emit_C_block(nc, d, t, p, att_t, sj, drain="act"):
    """One [128 s-rows, 2048] output block: 16 accumulating MMs, PSUM
    drained to bf16 (ACT, or alternating DVE/ACT when interleaved into
    the B stream so neither engine gates the pac PSUM rotation), one
    DMA out."""
    import concourse.mybir as mybir
    F32 = mybir.dt.float32
    BF16 = mybir.dt.bfloat16
    ob = p["oc"].tile([128, E], BF16, name="ob")
    for ocn in range(E // 512):
        ps = p["pac"].tile([128, 512], F32, name="psC", tag="pac")
        for hc in range(HPC):
            nc.tensor.matmul(ps[:],
                             att_t[hc][:, sj * 128:(sj + 1) * 128],
                             t["wo"][hc][:, ocn * 512:(ocn + 1) * 512],
                             start=(hc == 0), stop=(hc == HPC - 1))
        dst = ob[:, ocn * 512:(ocn + 1) * 512]
        if drain == "mix" and ocn % 2 == 0:
            nc.vector.tensor_copy(dst, ps[:])
        else:
            nc.scalar.copy(dst, ps[:])
    nc.sync.dma_start(d["out"][sj * 128:(sj + 1) * 128, :], ob[:])


def _emit_C_half(nc, tc, d, t, p, att_t, qp, skip=()):
    for sj in range(qp * 8, (qp + 1) * 8):
        if sj in skip:
            continue
        _emit_C_block(nc, d, t, p, att_t, sj)


def _emit_lat_loads(nc, d, t, p):
    """Load the gathered latents (fresh each iteration: their DMA is
    part of the per-iteration cost) into 8 [128, S] tiles."""
    import concourse.mybir as mybir
    BF16 = mybir.dt.bfloat16
    lkv, lq = [], []
    for l in range(NL):
        tl = p["lat"].tile([128, S], BF16, name=f"lkv{l}")
        nc.sync.dma_start(tl[:], d["latT"][l * 128:(l + 1) * 128, :])
        lkv.append(tl)
    for l in range(NL):
        tl = p["lat"].tile([128, S], BF16, name=f"lq{l}")
        nc.sync.dma_start(tl[:],
                          d["latT"][LAT + l * 128:LAT + (l + 1) * 128, :])
        lq.append(tl)
    t["lat_kv"], t["lat_q"] = lkv, lq


def _emit_body(nc, tc, d, t, p, att_t):
    _emit_lat_loads(nc, d, t, p)
    # A: merged passes; q-side inline for s-chunks 0,1 only
    xt2 = None
    for sc in range(NSC):
        xt2 = _emit_A_pass(nc, tc, d, t, p, sc, do_q=(sc < 2),
                           xt2=None if sc % 2 == 0 else xt2)

    def between0(h):
        # late q-side projections keep the PE fed while ACT runs exps
        if h == 1:
            _emit_q_side(nc, t, p, 2, on_act=False)
        elif h == 2:
            _emit_q_side(nc, t, p, 3, on_act=False)
        # (on_act=False: ACT is the B0 pacing engine, pac slots suffice)

    _emit_B_half(nc, tc, d, t, p, att_t, 0, between=between0)

    def between1(h):
        # interleave C half-0 blocks between B half-1 streams
        if h >= 1:
            _emit_C_block(nc, d, t, p, att_t, 2 * (h - 1), drain="mix")
            _emit_C_block(nc, d, t, p, att_t, 2 * (h - 1) + 1, drain="mix")

    _emit_B_half(nc, tc, d, t, p, att_t, 1, between=between1)
    _emit_C_block(nc, d, t, p, att_t, 6, drain="mix")
    _emit_C_block(nc, d, t, p, att_t, 7, drain="mix")
    _emit_C_half(nc, tc, d, t, p, att_t, 1)


def _build_program(loop=None):
    """loop=None: normal kernel. loop=(phase, n): benchmark variant with a
    hardware For_i loop repeating one phase (or the full body) n times."""
    import concourse.bacc as bacc
    import concourse.mybir as mybir
    import concourse.tile as tile

    nc = bacc.Bacc("TRN2", target_bir_lowering=False, debug=False,
                   num_devices=NCORES)
    d = _mk(nc)

    with tile.TileContext(nc) as tc, ExitStack() as top:
        t = _consts(nc, tc, top, d)
        att_t = t["Q"]  # att output overwrites consumed Q columns
        p = _pools(nc, tc, top)
        if loop is None:
            _emit_body(nc, tc, d, t, p, att_t)
        else:
            phase, n = loop

            def _loopctx():
                # n == 1: no hardware loop (lets TimelineSim run phases)
                from contextlib import nullcontext
                return tc.For_i(0, n, 1) if n > 1 else nullcontext()

            def _fill(tile_, w):
                nc.sync.dma_start(tile_[:], d["xT"][0:128, 0:w])

            if phase == "A":
                with _loopctx():
                    _emit_lat_loads(nc, d, t, p)
                    xt2 = None
                    for sc in range(NSC):
                        xt2 = _emit_A_pass(
                            nc, tc, d, t, p, sc, do_q=True,
                            xt2=None if sc % 2 == 0 else xt2)
            elif phase.startswith("B"):
                mode = {"B": "full", "B0": "qk", "B1": "qke",
                        "B2": "qkep", "B3": "qkea"}[phase]
                for h in range(HPC):
                    _fill(t["K"][h], S)
                    _fill(t["Q"][h], S)
                for i in range(NKC):
                    _fill(t["V"][i], HPC * D)
                with _loopctx():
                    for qp in range(2):
                        _emit_B_half(nc, tc, d, t, p, att_t, qp, mode)
            elif phase == "C":
                for h in range(HPC):
                    _fill(att_t[h], S)
                with _loopctx():
                    for qp in range(2):
                        _emit_C_half(nc, tc, d, t, p, att_t, qp)
            elif phase == "full":
                with _loopctx():
                    _emit_body(nc, tc, d, t, p, att_t)
            elif phase == "full2":
                # two unrolled iterations (steady-state sim estimation)
                _emit_body(nc, tc, d, t, p, att_t)
                _emit_body(nc, tc, d, t, p, att_t)
            else:
                raise ValueError(phase)

    nc.compile()
    return nc


def _rope_tables():
    inv_freq = 1.0 / (10000.0 ** (np.arange(0, R, 2, dtype=np.float64) / R))
    t = np.arange(S, dtype=np.float64)
    freqs = np.outer(t, inv_freq)                       # (S, R/2)
    emb = np.concatenate([freqs, freqs], axis=-1)       # (S, R)
    cos = np.cos(emb).astype(np.float32)                # (S, R)
    sin = np.sin(emb).astype(np.float32)
    perm = np.array([(j // 2) if j % 2 == 0 else (j // 2) + R // 2
                     for j in range(R)])
    sign = np.array([-1.0 if j % 2 == 0 else 1.0
                     for j in range(R)], dtype=np.float32)
    cos_p = cos[:, perm].T.copy()                       # (R, S)
    sin_p = (sin[:, perm] * sign[None, :]).T.copy()     # (R, S)
    cosT = np.concatenate([cos_p, cos_p], axis=0)       # (128, S)
    sinT = np.concatenate([sin_p, sin_p], axis=0)
    return cosT, sinT, perm


def _bf16():
    import concourse.mybir as mybir
    return mybir.dt.np(mybir.dt.bfloat16)


def _per_core_inputs(inputs, core):
    b, hg = divmod(core, HPC)
    cosT, sinT, perm = _rope_tables()
    hsl64 = np.concatenate([hg * HPC * R + h * R + perm
                            for h in range(HPC)])       # permuted rope rows
    hs64 = slice(hg * HPC * R, (hg + 1) * HPC * R)      # natural 64-rows
    hs128 = slice(hg * HPC * D, (hg + 1) * HPC * D)     # natural 128-rows

    x = np.asarray(inputs["x"], dtype=np.float32)
    f = np.float32
    bf = _bf16()

    def c(a, dt=None):
        return np.ascontiguousarray(a).astype(dt if dt is not None else bf)

    g = {k: np.asarray(v, f) for k, v in inputs.items()}
    # biases: the latent down-projection biases fold into the up biases
    bk1 = g["bk_u"][hs64] + g["Wk_u"][hs64] @ g["bkv_d"]
    bv = g["bv_u"][hs128] + g["Wv_u"][hs128] @ g["bkv_d"]
    bq1 = g["bq_u"][hs64] + g["Wq_u"][hs64] @ g["bq_d"]
    bqr = (g["brq"] + g["Wrq"] @ g["bq_d"])[hsl64]

    lat = _LAT_CACHE.get(b)
    if lat is None:
        lat = np.zeros((2 * LAT, S), dtype=_bf16())

    im = {
        "xT": c(x[b].T),
        "latT": np.ascontiguousarray(lat).astype(_bf16()),
        "wk1u": c(g["Wk_u"][hs64].T),             # (LAT, 256)
        "wq1u": c(g["Wq_u"][hs64].T),
        "wqru": c(g["Wrq"][hsl64].T),             # (LAT, 256), rope-perm
        "wrkT": c(g["Wrk"][hsl64].T),
        "wvu": c(g["Wv_u"][hs128].T),             # (LAT, 512)
        "woT": c(g["Wo"].T[hs128]),
        "bk1f": c(bk1.reshape(2, 128).T, f),
        "bq1f": c(bq1.reshape(2, 128).T, f),
        "bqrf": c(bqr.reshape(2, 128).T, f),
        "brk": c(g["brk"][hsl64].reshape(2, 128).T, f),
        "bvf": c(bv.reshape(1, HPC * D), f),
        "onesd": np.ones((128, 1), dtype=bf),
        "cosT": cosT.astype(bf),
        "sinT": sinT.astype(bf),
    }
    return im


def _per_core_inputs_down(inputs, core):
    """Down-program inputs: x (its batch) + the core's latent shard
    weights (128 rows of Wkv_d and of Wq_d, by core index in the batch
    group)."""
    b, gsh = divmod(core, HPC)
    f = np.float32
    bf = _bf16()
    x = np.asarray(inputs["x"], dtype=f)
    wkv = np.asarray(inputs["Wkv_d"], dtype=f)[gsh * 128:(gsh + 1) * 128]
    wq = np.asarray(inputs["Wq_d"], dtype=f)[gsh * 128:(gsh + 1) * 128]
    wdn = np.concatenate([wkv.T, wq.T], axis=1)   # (E, 256)
    return {
        "xT": np.ascontiguousarray(x[b].T).astype(bf),
        "wdn": np.ascontiguousarray(wdn).astype(bf),
    }


def _gather_latents(down_res):
    """Concatenate the 4 per-core shards of each batch into full
    kv_d/q_d latents; fills _LAT_CACHE."""
    for b in range(B):
        kv = np.concatenate(
            [down_res[b * HPC + gsh]["lat"][0:128] for gsh in range(HPC)],
            axis=0)
        q = np.concatenate(
            [down_res[b * HPC + gsh]["lat"][128:256] for gsh in range(HPC)],
            axis=0)
        _LAT_CACHE[b] = np.concatenate([kv, q], axis=0)  # (2*LAT, S)


def _get_runtime(loop=None, donate=True):
    key = (loop, donate)
    if key in _RT:
        return _RT[key]
    import jax
    import numpy as _np
    from jax.sharding import Mesh, PartitionSpec
    from jax.experimental.shard_map import shard_map

    import concourse.mybir as mybir
    from concourse import bass2jax

    if loop is not None and loop[0] == "down":
        nc = _build_down(loop[1])
    else:
        nc = _build_program(loop)
    bass2jax.install_neuronx_cc_hook()

    partition_name = (nc.partition_id_tensor.name
                      if nc.partition_id_tensor else None)
    in_names, out_names, out_avals, zero_shapes = [], [], [], []
    for alloc in nc.m.functions[0].allocations:
        if not isinstance(alloc, mybir.MemoryLocationSet):
            continue
        name = alloc.memorylocations[0].name
        if alloc.kind == "ExternalInput":
            if name != partition_name:
                in_names.append(name)
        elif alloc.kind == "ExternalOutput":
            out_names.append(name)
            np_dt = mybir.dt.np(alloc.dtype)
            out_avals.append(jax.core.ShapedArray(
                tuple(alloc.tensor_shape), np_dt))
            zero_shapes.append((tuple(alloc.tensor_shape), np_dt))

    n_params = len(in_names)
    n_outs = len(out_names)
    all_in_names = list(in_names) + list(out_names)
    if partition_name is not None:
        all_in_names.append(partition_name)

    def _body(*args):
        operands = list(args)
        if partition_name is not None:
            operands.append(bass2jax.partition_id_tensor())
        outs = bass2jax._bass_exec_p.bind(
            *operands,
            out_avals=tuple(out_avals),
            in_names=tuple(all_in_names),
            out_names=tuple(out_names),
            lowering_input_output_aliases=(),
            sim_require_finite=True,
            sim_require_nnan=True,
            nc=nc,
        )
        return tuple(outs)

    devices = jax.devices()[:NCORES]
    mesh = Mesh(_np.asarray(devices), ("core",))
    in_specs = (PartitionSpec("core"),) * (n_params + n_outs)
    out_specs = (PartitionSpec("core"),) * n_outs
    donate_idx = (tuple(range(n_params, n_params + n_outs)) if donate
                  else ())
    sharded = jax.jit(
        shard_map(_body, mesh=mesh, in_specs=in_specs, out_specs=out_specs,
                  check_rep=False),
        donate_argnums=donate_idx, keep_unused=True)

    _RT[key] = dict(sharded=sharded, in_names=in_names, out_names=out_names,
                    zero_shapes=zero_shapes, n_outs=n_outs)
    return _RT[key]


def _run_cores(in_maps, loop=None):
    rt = _get_runtime(loop)
    import numpy as _np
    concat_in = [
        _np.concatenate([in_maps[c][name] for c in range(NCORES)], axis=0)
        for name in rt["in_names"]
    ]
    concat_zeros = [
        _np.zeros((NCORES * shp[0],) + shp[1:], dt)
        for (shp, dt) in rt["zero_shapes"]
    ]
    out_arrs = rt["sharded"](*concat_in, *concat_zeros)
    res = []
    for c in range(NCORES):
        m = {}
        for i, name in enumerate(rt["out_names"]):
            shp, dt = rt["zero_shapes"][i]
            m[name] = _np.asarray(out_arrs[i]).reshape((NCORES,) + shp)[c]
        res.append(m)
    return res


def kernel(**inputs):
    # program 1: tensor-parallel latent down-projections; host gathers
    # the shards into full latents for each batch group
    down_maps = [_per_core_inputs_down(inputs, c) for c in range(NCORES)]
    down_res = _run_cores(down_maps, loop=("down", 1))
    _gather_latents(down_res)

    # program 2: up-projections from the latents, attention, out-proj
    in_maps = [_per_core_inputs(inputs, c) for c in range(NCORES)]
    res = _run_cores(in_maps)
    bo = np.asarray(inputs["bo"], dtype=np.float32)
    final = np.empty((B, S, E), dtype=np.float32)
    for b in range(B):
        acc = res[HPC * b]["out"].astype(np.float32)
        for g in range(1, HPC):
            acc = acc + res[HPC * b + g]["out"].astype(np.float32)
        final[b] = acc + bo[None, :]
    return final


# revision 53
# speedup vs baseline: 1.0754x; 1.0754x over previous
"""MultiHeadLatentAttention TRN2 kernel (two-program, latent-TP).

Sharding: 8 cores = 2 (batch) x 4 (head groups of 4 heads).  Two device
programs with a host-side latent relay between them:

Program 1 (down): the shared latent down-projections (Wkv_d, Wq_d) are
tensor-parallel split across the 4 cores of each batch group - each core
computes a distinct 128-row shard of kv_d AND q_d over all S (16-deep
E-contraction, x resident as contiguous [128, S] e-chunk tiles).  The
host concatenates the shards into full [2*LAT, S] latents per batch
(pure layout, no host compute) - this is what makes MLA's low-rank
structure pay: every core then contracts over LAT=512 instead of E=2048
for the k1/q1/qr/v up-projections, cutting per-core matmul count ~30%
versus folding W_up @ W_down into direct E->head projections.

Program 2 (main): per core, for its batch and its 4 heads: up-project
K^T/Q^T (with RoPE; rope-k comes straight from x) and V from the
latents, full attention, and a partial output projection (its 512 rows
of Wo's input dim).  bf16 partial outputs are summed on the host in
fp32 (+ bo).  Down-projection biases are folded into the up biases.

All matmul operands bf16 (fp32 PSUM accumulation).  Schedule is built
around the PE array (the global bottleneck), keeping the ACT exp stream
and DVE row-accum off the critical path:

  A: per s-chunk: k1/V from latents, rope-k from x; q-side for s-chunks
     0,1 inline.  PSUM bias-consumers run on ACT (idle here) so the
     2-slot pac PSUM rotation never waits on the DVE stream.
  B half 0: 4 head streams; q-side projections for s-chunks 2,3 are
     interleaved between streams as exp-independent PE filler.
  B half 1: 4 head streams with all 8 C-half-0 output blocks
     interleaved (PSUM drains alternate DVE/ACT); C half 1 at the end
     overlaps the next iteration's DMA/DVE leads in the For_i loop.

  B stream: QK (2 MM) -> exp on ACT -> bf16 row-accum on DVE -> PV
  trailing by LAG chunks; PV accumulators drain to bf16 att tiles
  (overwriting consumed Q columns) on DVE.  Softmax tail (ones-matmul
  row sums on PE, reciprocal+normalize on DVE, partition-broadcast on
  Pool) is software-pipelined 1-2 streams behind.  Softmax skips
  max-subtraction (scores are bounded).
"""
import sys

sys.path.insert(0, "/opt/trn_rl_repo")

from contextlib import ExitStack

import numpy as np

H = 16
E = 2048
LAT = E // 4          # 512
D = E // H            # 128
R = D // 2            # 64
B, S = 2, 2048
HPC = H // 4          # 4 heads per core
NCORES = 8
NE = E // 128         # 16 contraction chunks over E
NL = LAT // 128       # 4 contraction chunks over LAT
SW = 512              # s-chunk width for projections
NSC = S // SW         # 4 s-chunks
NKC = S // 128        # 16 key chunks
SCALE = 1.0 / float(np.sqrt(D))
LAG = 5               # PV trails QK/exp by LAG k-chunks

_RT = {}  # cached runtimes
_LAT_CACHE = {}  # batch -> gathered latents (bf16 [2*LAT, S]), set by kernel()


def _mk_down(nc):
    """Down-program DRAM I/O: x, the core's latent-shard weights, lat out.

    Each core computes a distinct 128-row shard of kv_d AND of q_d for
    its batch, over all S (tensor-parallel split of the shared latent
    down-projections across the 4 cores of a batch group; the host
    gathers the shards between the two programs)."""
    import concourse.mybir as mybir
    BF16 = mybir.dt.bfloat16
    d = {}
    d["xT"] = nc.dram_tensor("xT", [E, S], BF16, kind="ExternalInput")
    # [E, 256]: cols 0:128 = kv_d shard rows, 128:256 = q_d shard rows
    d["wdn"] = nc.dram_tensor("wdn", [E, 256], BF16, kind="ExternalInput")
    d["lat"] = nc.dram_tensor("lat", [256, S], BF16, kind="ExternalOutput")
    return d


def _emit_down_body(nc, tc, d, wdn, p):
    """x resident as 16 contiguous [128, S] e-chunk tiles (full-row DMA,
    double-buffered across iterations), then 4 s-chunk passes of two
    128-row latent projections (16-deep E-contraction), ACT-drained to
    bf16, DMA'd out."""
    import concourse.mybir as mybir
    F32 = mybir.dt.float32
    BF16 = mybir.dt.bfloat16
    xe = []
    for e in range(NE):
        tl = p["xa"].tile([128, S], BF16, name=f"xe{e}")
        nc.sync.dma_start(tl[:], d["xT"][e * 128:(e + 1) * 128, :])
        xe.append(tl)
    for sc in range(NSC):
        ssl = slice(sc * SW, (sc + 1) * SW)
        for m in range(2):
            ps = p["pac"].tile([128, SW], F32, name="psD", tag="pac")
            for e in range(NE):
                nc.tensor.matmul(
                    ps[:], wdn[:, e * 256 + m * 128:e * 256 + (m + 1) * 128],
                    xe[e][:, ssl],
                    start=(e == 0), stop=(e == NE - 1))
            ob = p["oc"].tile([128, SW], BF16, name="obd")
            nc.scalar.copy(ob[:], ps[:])
            nc.sync.dma_start(d["lat"][m * 128:(m + 1) * 128, ssl], ob[:])


def _build_down(loop_n=None):
    """The down-projection program (prog 1).  loop_n: For_i repeat count
    for benchmarking (None = single pass)."""
    import concourse.bacc as bacc
    import concourse.mybir as mybir
    import concourse.tile as tile

    BF16 = mybir.dt.bfloat16
    nc = bacc.Bacc("TRN2", target_bir_lowering=False, debug=False,
                   num_devices=NCORES)
    d = _mk_down(nc)
    with tile.TileContext(nc) as tc, ExitStack() as top:
        wpool = top.enter_context(tc.tile_pool(name="wp", bufs=1))
        wdn = wpool.tile([128, NE * 256], BF16, name="wdn_t")
        nc.sync.dma_start(
            wdn[:].rearrange("p (e c) -> p e c", e=NE),
            d["wdn"][:].rearrange("(e p) c -> p e c", p=128))
        p = {}
        p["xa"] = top.enter_context(tc.tile_pool(name="xa", bufs=2))
        p["oc"] = top.enter_context(tc.tile_pool(name="oc", bufs=4))
        p["pac"] = top.enter_context(tc.tile_pool(name="pac", bufs=2,
                                                  space="PSUM"))
        if loop_n is None or loop_n == 1:
            _emit_down_body(nc, tc, d, wdn, p)
        else:
            # two body copies per trip: halves any loop back-edge drain
            # while the measured per-iteration slope stays exact
            assert loop_n % 2 == 0
            with tc.For_i(0, loop_n // 2, 1):
                _emit_down_body(nc, tc, d, wdn, p)
                _emit_down_body(nc, tc, d, wdn, p)
    nc.compile()
    return nc


def _mk(nc):
    """Declare DRAM I/O; returns dict of handles."""
    import concourse.mybir as mybir
    F32 = mybir.dt.float32
    BF16 = mybir.dt.bfloat16
    d = {}
    d["xT"] = nc.dram_tensor("xT", [E, S], BF16, kind="ExternalInput")
    d["latT"] = nc.dram_tensor("latT", [2 * LAT, S], BF16,
                               kind="ExternalInput")
    for nm in ("wk1u", "wq1u", "wqru"):
        d[nm] = nc.dram_tensor(nm, [LAT, HPC * R], BF16,
                               kind="ExternalInput")
    d["wvu"] = nc.dram_tensor("wvu", [LAT, HPC * D], BF16,
                              kind="ExternalInput")
    d["wrkT"] = nc.dram_tensor("wrkT", [E, HPC * R], BF16,
                               kind="ExternalInput")
    d["woT"] = nc.dram_tensor("woT", [HPC * D, E], BF16,
                              kind="ExternalInput")
    for nm in ("bk1f", "bq1f", "bqrf", "brk"):
        d[nm] = nc.dram_tensor(nm, [128, 2], F32, kind="ExternalInput")
    d["bvf"] = nc.dram_tensor("bvf", [1, HPC * D], F32,
                              kind="ExternalInput")
    d["onesd"] = nc.dram_tensor("onesd", [128, 1], BF16,
                                kind="ExternalInput")
    d["cosT"] = nc.dram_tensor("cosT", [128, S], BF16,
                               kind="ExternalInput")
    d["sinT"] = nc.dram_tensor("sinT", [128, S], BF16,
                               kind="ExternalInput")
    d["out"] = nc.dram_tensor("out", [S, E], BF16, kind="ExternalOutput")
    return d


def _consts(nc, tc, top, d):
    """Persistent tiles: K/Q/V storage, biases, ones, weights, cos/sin."""
    import concourse.mybir as mybir
    F32 = mybir.dt.float32
    BF16 = mybir.dt.bfloat16

    kq_pool = top.enter_context(tc.tile_pool(name="kq", bufs=1))
    v_pool = top.enter_context(tc.tile_pool(name="vp", bufs=1))
    cpool = top.enter_context(tc.tile_pool(name="cp", bufs=1))
    wpool = top.enter_context(tc.tile_pool(name="wp", bufs=1))

    t = {}
    t["K"] = [kq_pool.tile([128, S], BF16, name=f"Kt{h}") for h in range(HPC)]
    t["Q"] = [kq_pool.tile([128, S], BF16, name=f"Qt{h}") for h in range(HPC)]
    t["V"] = [v_pool.tile([128, HPC * D], BF16, name=f"Vt{i}")
              for i in range(NKC)]

    def ld(name, dram, shape, dt=F32):
        tl = cpool.tile(shape, dt, name=name)
        nc.sync.dma_start(tl[:], dram[:])
        return tl

    t["ones"] = ld("ones_t", d["onesd"], [128, 1], BF16)
    t["bk1f"] = ld("bk1f_t", d["bk1f"], [128, 2])
    t["bq1f"] = ld("bq1f_t", d["bq1f"], [128, 2])
    t["bqrf"] = ld("bqrf_t", d["bqrf"], [128, 2])
    t["brk"] = ld("brk_t", d["brk"], [128, 2])
    t["cos"] = ld("cos_t", d["cosT"], [128, S], BF16)
    t["sin"] = ld("sin_t", d["sinT"], [128, S], BF16)
    bvf_row = ld("bvf_row", d["bvf"], [1, HPC * D])
    bvf_bc = cpool.tile([128, HPC * D], F32, name="bvf_bc")
    nc.gpsimd.partition_broadcast(bvf_bc[:], bvf_row[:])
    t["bvf_bc"] = bvf_bc

    # projection weights, loaded once, ordered by first use so the first
    # A pass isn't blocked behind later weights on the DMA queue.
    # Latent up-projections fold contraction as (l p) -> p l (NL=4
    # chunks); rk keeps the E-deep fold (NE=16 chunks).
    dnw = {}
    for nm, key, nch, cw in (("k1u", "wk1u", NL, HPC * R),
                             ("rk", "wrkT", NE, HPC * R),
                             ("vu", "wvu", NL, HPC * D),
                             ("q1u", "wq1u", NL, HPC * R),
                             ("qru", "wqru", NL, HPC * R)):
        tl = wpool.tile([128, nch * cw], BF16, name=f"wd{nm}")
        nc.sync.dma_start(
            tl[:].rearrange("p (e c) -> p e c", e=nch),
            d[key][:].rearrange("(e p) c -> p e c", p=128))
        dnw[nm] = (tl, cw)
    t["dnw"] = dnw

    # output projection weights
    t["wo"] = [wpool.tile([128, E], BF16, name=f"wo{hc}")
               for hc in range(HPC)]
    for hc in range(HPC):
        nc.sync.dma_start(t["wo"][hc][:], d["woT"][hc * 128:(hc + 1) * 128, :])
    return t


def _pools(nc, tc, st):
    import concourse.mybir as mybir  # noqa: F401
    p = {}
    p["xa"] = st.enter_context(tc.tile_pool(name="xa", bufs=2))
    p["lat"] = st.enter_context(tc.tile_pool(name="lat", bufs=1))
    p["rp"] = st.enter_context(tc.tile_pool(name="rp", bufs=1))
    p["pe"] = st.enter_context(tc.tile_pool(name="pe", bufs=3))
    p["cb"] = st.enter_context(tc.tile_pool(name="cb", bufs=2))
    p["oc"] = st.enter_context(tc.tile_pool(name="oc", bufs=2))
    # PSUM: pac 2x[128,512] (A chains, C groups, row-sum outputs),
    # psS 2x[128,1024] (score tiles), psO 2x[128,512] (PV accumulators)
    p["pac"] = st.enter_context(tc.tile_pool(name="pac", bufs=2,
                                             space="PSUM"))
    p["psS"] = st.enter_context(tc.tile_pool(name="psS", bufs=2,
                                             space="PSUM"))
    p["psO"] = st.enter_context(tc.tile_pool(name="psO", bufs=1,
                                             space="PSUM"))
    return p


def _proj(nc, t, p, wname, m, xt):
    """x @ W chunk: 16-deep contraction over E, [128, SW] psum out."""
    import concourse.mybir as mybir
    F32 = mybir.dt.float32
    wt, cw = t["dnw"][wname]
    ps = p["pac"].tile([128, SW], F32, name="psA", tag="pac")
    for e in range(NE):
        nc.tensor.matmul(
            ps[:], wt[:, e * cw + m * 128:e * cw + (m + 1) * 128],
            xt[:, e * SW:(e + 1) * SW],
            start=(e == 0), stop=(e == NE - 1))
    return ps


def _proj_lat(nc, t, p, wname, m, lats, ssl, dst=None):
    """latent @ W_up chunk: 4-deep contraction over LAT, [128, SW] out.

    dst=(tile, col) accumulates into a half of a wider PSUM tile (the
    psS score tiles are idle during the A section - using their 4 banks
    as extra rotation slots keeps the PE from stalling on the 2-slot pac
    pool's consumer latency)."""
    import concourse.mybir as mybir
    F32 = mybir.dt.float32
    wt, cw = t["dnw"][wname]
    if dst is None:
        ps = p["pac"].tile([128, SW], F32, name="psA", tag="pac")
        out = ps[:]
    else:
        ps, col = dst
        out = ps[:, col:col + SW]
    for l in range(NL):
        nc.tensor.matmul(
            out, wt[:, l * cw + m * 128:l * cw + (m + 1) * 128],
            lats[l][:, ssl],
            start=(l == 0), stop=(l == NL - 1))
    return None if dst is not None else ps


def _rope(nc, t, p, ps_ap, bias_t, m, dst, ssl, rope_on_act=True):
    """RoPE rows: ps [128 = 2 heads x 64 rope rows, SW] -> dst rows R:D.

    The PSUM consumer (bias add) runs on ACT (idle during projections)
    so the pac slot frees without waiting on the DVE stream; in B half 0
    (ACT busy with exps) it goes to DVE instead."""
    import concourse.mybir as mybir
    from concourse.alu_op_type import AluOpType
    BF16 = mybir.dt.bfloat16
    AF = mybir.ActivationFunctionType
    swap_mask = [i ^ 1 for i in range(32)]
    cos_s, sin_s = t["cos"][:, ssl], t["sin"][:, ssl]
    xb = p["rp"].tile([128, SW], BF16, name="xb")
    if rope_on_act:
        nc.scalar.activation(xb[:], ps_ap, AF.Identity,
                             bias=bias_t[:, m:m + 1])
    else:
        nc.vector.tensor_scalar_add(xb[:], ps_ap, bias_t[:, m:m + 1])
    sh = p["rp"].tile([128, SW], BF16, name="sh")
    nc.vector.stream_shuffle(sh[:], xb[:], swap_mask)
    t1 = p["rp"].tile([128, SW], BF16, name="t1")
    nc.vector.tensor_tensor(t1[:], xb[:], cos_s, op=AluOpType.mult)
    t2 = p["rp"].tile([128, SW], BF16, name="t2")
    nc.vector.tensor_tensor(t2[:], sh[:], sin_s, op=AluOpType.mult)
    nc.vector.tensor_tensor(dst[2 * m][R:D, ssl], t1[0:R, :],
                            t2[0:R, :], op=AluOpType.add)
    nc.vector.tensor_tensor(dst[2 * m + 1][R:D, ssl], t1[R:D, :],
                            t2[R:D, :], op=AluOpType.add)


def _emit_q_side(nc, t, p, sc, on_act=True):
    """q1 + rope-q projections for s-chunk sc from the q latents.

    on_act=False routes the PSUM bias-consumers to DVE instead - used
    for the q-sides interleaved into B half 0, where ACT is the pacing
    engine (the exp stream) but DVE has slack."""
    ssl = slice(sc * SW, (sc + 1) * SW)
    Q_t = t["Q"]
    lq = t["lat_q"]
    import concourse.mybir as mybir
    AF = mybir.ActivationFunctionType
    for m in range(2):
        ps = _proj_lat(nc, t, p, "q1u", m, lq, ssl)
        nc.scalar.activation(Q_t[2 * m][0:R, ssl], ps[0:R, :],
                             AF.Identity, bias=t["bq1f"][0:R, m:m + 1])
        nc.scalar.activation(Q_t[2 * m + 1][0:R, ssl], ps[R:D, :],
                             AF.Identity, bias=t["bq1f"][R:D, m:m + 1])
        ps = _proj_lat(nc, t, p, "qru", m, lq, ssl)
        _rope(nc, t, p, ps[:], t["bqrf"], m, Q_t, ssl)


def _emit_A_pass(nc, tc, d, t, p, sc, do_q):
    """One merged pass for s-chunk sc: xt load (rk only), k1/V from the
    kv latents, rope-k from x (+ q-side from the q latents if do_q)."""
    import concourse.mybir as mybir
    from concourse.alu_op_type import AluOpType
    F32 = mybir.dt.float32
    BF16 = mybir.dt.bfloat16
    K_t, V_t = t["K"], t["V"]
    lkv = t["lat_kv"]
    ssl = slice(sc * SW, (sc + 1) * SW)

    xt = p["xa"].tile([128, NE * SW], BF16, name="xt")
    nc.sync.dma_start(
        xt[:].rearrange("p (e s) -> p e s", e=NE),
        d["xT"][:, ssl].rearrange("(e p) s -> p e s", p=128))

    AF = mybir.ActivationFunctionType
    for m in range(2):  # k1 -> K rows 0..63
        ps = _proj_lat(nc, t, p, "k1u", m, lkv, ssl)
        nc.scalar.activation(K_t[2 * m][0:R, ssl], ps[0:R, :], AF.Identity,
                             bias=t["bk1f"][0:R, m:m + 1])
        nc.scalar.activation(K_t[2 * m + 1][0:R, ssl], ps[R:D, :],
                             AF.Identity, bias=t["bk1f"][R:D, m:m + 1])
    for m in range(2):  # rope-k from x
        ps = _proj(nc, t, p, "rk", m, xt)
        _rope(nc, t, p, ps[:], t["brk"], m, K_t, ssl)
    for j in range(SW // 128):  # V (s, feat) layout, 4-deep over LAT
        wt, cw = t["dnw"]["vu"]
        ps = p["pac"].tile([128, HPC * D], F32, name="psA", tag="pac")
        for l in range(NL):
            nc.tensor.matmul(
                ps[:], lkv[l][:, sc * SW + j * 128:sc * SW + (j + 1) * 128],
                wt[:, l * cw:(l + 1) * cw],
                start=(l == 0), stop=(l == NL - 1))
        nc.vector.tensor_tensor(V_t[sc * (SW // 128) + j][:], ps[:],
                                t["bvf_bc"][:], op=AluOpType.add)
    if do_q:
        _emit_q_side(nc, t, p, sc)


def _emit_B_half(nc, tc, d, t, p, att_t, qp, mode="full", between=None):
    import concourse.mybir as mybir
    from concourse.alu_op_type import AluOpType
    F32 = mybir.dt.float32
    BF16 = mybir.dt.bfloat16
    AF = mybir.ActivationFunctionType
    K_t, Q_t, V_t = t["K"], t["Q"], t["V"]

    def stream(h):
        """QK/exp/row-acc/PV for head h; returns deferred-tail closures.

        The softmax tail (row-sum matmuls, 1/r, broadcast, normalize) is
        deferred 1-2 streams so its small ops land BEHIND the next
        stream's work in each engine's in-order queue.  PV accumulators
        drain to SBUF (unnormalized bf16 att tiles) on Pool as soon as
        PV finishes, freeing the PSUM slots.
        """
        qa = slice(qp * 1024, qp * 1024 + 512)
        qb = slice(qp * 1024 + 512, (qp + 1) * 1024)
        oA = p["psO"].tile([128, 512], F32, name="oA")
        oB = p["psO"].tile([128, 512], F32, name="oB")
        acc0 = p["pe"].tile([128, 1024], BF16, name="acc0", bufs=2)
        acc1 = p["pe"].tile([128, 1024], BF16, name="acc1", bufs=2)
        accs = (acc0, acc1)
        pes = {}

        def pv(kk):
            pet = pes.pop(kk)
            nc.tensor.matmul(oA[:], V_t[kk][:, h * D:(h + 1) * D],
                             pet[:, 0:512], start=(kk == 0),
                             stop=(kk == NKC - 1))
            nc.tensor.matmul(oB[:], V_t[kk][:, h * D:(h + 1) * D],
                             pet[:, 512:1024], start=(kk == 0),
                             stop=(kk == NKC - 1))

        for kk in range(NKC):
            ksl = slice(kk * 128, (kk + 1) * 128)
            pp = p["psS"].tile([128, 1024], F32, name="pp")
            nc.tensor.matmul(pp[:, 0:512], K_t[h][:, ksl], Q_t[h][:, qa],
                             start=True, stop=True)
            nc.tensor.matmul(pp[:, 512:1024], K_t[h][:, ksl],
                             Q_t[h][:, qb], start=True, stop=True)
            if mode == "qk":
                continue
            pet = p["pe"].tile([128, 1024], BF16, name="pet", bufs=8)
            nc.scalar.activation(pet[:], pp[:], AF.Exp, scale=SCALE)
            if mode == "qke":
                continue
            if mode != "qkep":
                acc = accs[kk % 2]
                if kk < 2:
                    nc.vector.tensor_copy(acc[:], pet[:])
                else:
                    nc.vector.tensor_tensor(acc[:], pet[:], acc[:],
                                            op=AluOpType.add)
            if mode != "qkea":
                pes[kk] = pet
                if kk >= LAG:
                    pv(kk - LAG)
        if mode != "full":
            return None
        for kk in range(NKC - LAG, NKC):
            pv(kk)
        # drain PV accumulators (f32 PSUM -> bf16 att SBUF) on DVE
        nc.vector.tensor_copy(att_t[h][:, qa], oA[:])
        nc.vector.tensor_copy(att_t[h][:, qb], oB[:])
        nc.vector.tensor_tensor(acc0[:], acc1[:], acc0[:],
                                op=AluOpType.add)

        def tail1():
            sumA = p["pac"].tile([1, 512], F32, name="sumA", tag="pac")
            nc.tensor.matmul(sumA[:], t["ones"][:], acc0[:, 0:512],
                             start=True, stop=True)
            sumB = p["pac"].tile([1, 512], F32, name="sumB", tag="pac")
            nc.tensor.matmul(sumB[:], t["ones"][:], acc0[:, 512:1024],
                             start=True, stop=True)
            ci = p["cb"].tile([1, 1024], BF16, name="ci")
            with nc.allow_low_precision("softmax denom recip in bf16"):
                nc.vector.reciprocal(ci[:, 0:512], sumA[:])
                nc.vector.reciprocal(ci[:, 512:1024], sumB[:])
            return ci

        def tail2(ci):
            bcT = p["cb"].tile([128, 1024], BF16, name="bcT")
            nc.gpsimd.partition_broadcast(bcT[:], ci[:])
            nc.vector.tensor_tensor(att_t[h][:, qa], att_t[h][:, qa],
                                    bcT[:, 0:512], op=AluOpType.mult)
            nc.vector.tensor_tensor(att_t[h][:, qb], att_t[h][:, qb],
                                    bcT[:, 512:1024], op=AluOpType.mult)

        return tail1, tail2

    # 3-stage software pipeline: stream h's row-sum+recip lands behind
    # stream h+1; its broadcast+normalize behind stream h+2.
    from collections import deque
    pend = deque()  # [tail1, tail2, ci]
    for h in range(HPC):
        if between is not None:
            between(h)
        pair = stream(h)
        if pair is None:
            continue
        pend.append(list(pair) + [None])
        if len(pend) >= 2:
            e = pend[-2]
            e[2] = e[0]()
        if len(pend) >= 3:
            e = pend.popleft()
            if e[2] is None:
                e[2] = e[0]()
            e[1](e[2])
    while pend:
        e = pend.popleft()
        if e[2] is None:
            e[2] = e[0]()
        e[1](e[2])


def _emit_C_block(nc, d, t, p, att_t, sj, drain="act"):
    """One [128 s-rows, 2048] output block: 16 accumulating MMs, PSUM
    drained to bf16 (ACT, or alternating DVE/ACT when interleaved into
    the B stream so neither engine gates the pac PSUM rotation), one
    DMA out."""
    import concourse.mybir as mybir
    F32 = mybir.dt.float32
    BF16 = mybir.dt.bfloat16
    ob = p["oc"].tile([128, E], BF16, name="ob")
    for ocn in range(E // 512):
        ps = p["pac"].tile([128, 512], F32, name="psC", tag="pac")
        for hc in range(HPC):
            nc.tensor.matmul(ps[:],
                             att_t[hc][:, sj * 128:(sj + 1) * 128],
                             t["wo"][hc][:, ocn * 512:(ocn + 1) * 512],
                             start=(hc == 0), stop=(hc == HPC - 1))
        dst = ob[:, ocn * 512:(ocn + 1) * 512]
        if drain == "mix" and ocn % 2 == 0:
            nc.vector.tensor_copy(dst, ps[:])
        else:
            nc.scalar.copy(dst, ps[:])
    nc.sync.dma_start(d["out"][sj * 128:(sj + 1) * 128, :], ob[:])


def _emit_C_half(nc, tc, d, t, p, att_t, qp, skip=()):
    for sj in range(qp * 8, (qp + 1) * 8):
        if sj in skip:
            continue
        _emit_C_block(nc, d, t, p, att_t, sj)


def _emit_lat_loads(nc, d, t, p):
    """Load the gathered latents (fresh each iteration: their DMA is
    part of the per-iteration cost) into 8 [128, S] tiles."""
    import concourse.mybir as mybir
    BF16 = mybir.dt.bfloat16
    lkv, lq = [], []
    for l in range(NL):
        tl = p["lat"].tile([128, S], BF16, name=f"lkv{l}")
        nc.sync.dma_start(tl[:], d["latT"][l * 128:(l + 1) * 128, :])
        lkv.append(tl)
    for l in range(NL):
        tl = p["lat"].tile([128, S], BF16, name=f"lq{l}")
        nc.sync.dma_start(tl[:],
                          d["latT"][LAT + l * 128:LAT + (l + 1) * 128, :])
        lq.append(tl)
    t["lat_kv"], t["lat_q"] = lkv, lq


def _emit_body(nc, tc, d, t, p, att_t):
    _emit_lat_loads(nc, d, t, p)
    # A: merged passes; q-side inline for s-chunks 0,1 only
    for sc in range(NSC):
        _emit_A_pass(nc, tc, d, t, p, sc, do_q=(sc < 2))

    def between0(h):
        # late q-side projections keep the PE fed while ACT runs exps
        if h == 1:
            _emit_q_side(nc, t, p, 2, on_act=False)
        elif h == 2:
            _emit_q_side(nc, t, p, 3, on_act=False)
        # (on_act=False: ACT is the B0 pacing engine, pac slots suffice)

    _emit_B_half(nc, tc, d, t, p, att_t, 0, between=between0)

    def between1(h):
        # interleave C half-0 blocks between B half-1 streams
        if h >= 1:
            _emit_C_block(nc, d, t, p, att_t, 2 * (h - 1), drain="mix")
            _emit_C_block(nc, d, t, p, att_t, 2 * (h - 1) + 1, drain="mix")

    _emit_B_half(nc, tc, d, t, p, att_t, 1, between=between1)
    _emit_C_block(nc, d, t, p, att_t, 6, drain="mix")
    _emit_C_block(nc, d, t, p, att_t, 7, drain="mix")
    _emit_C_half(nc, tc, d, t, p, att_t, 1)


def _build_program(loop=None):
    """loop=None: normal kernel. loop=(phase, n): benchmark variant with a
    hardware For_i loop repeating one phase (or the full body) n times."""
    import concourse.bacc as bacc
    import concourse.mybir as mybir
    import concourse.tile as tile

    nc = bacc.Bacc("TRN2", target_bir_lowering=False, debug=False,
                   num_devices=NCORES)
    d = _mk(nc)

    with tile.TileContext(nc) as tc, ExitStack() as top:
        t = _consts(nc, tc, top, d)
        att_t = t["Q"]  # att output overwrites consumed Q columns
        p = _pools(nc, tc, top)
        if loop is None:
            _emit_body(nc, tc, d, t, p, att_t)
        else:
            phase, n = loop

            def _loopctx():
                # n == 1: no hardware loop (lets TimelineSim run phases)
                from contextlib import nullcontext
                return tc.For_i(0, n, 1) if n > 1 else nullcontext()

            def _fill(tile_, w):
                nc.sync.dma_start(tile_[:], d["xT"][0:128, 0:w])

            if phase == "A":
                with _loopctx():
                    _emit_lat_loads(nc, d, t, p)
                    for sc in range(NSC):
                        _emit_A_pass(nc, tc, d, t, p, sc, do_q=True)
            elif phase.startswith("B"):
                mode = {"B": "full", "B0": "qk", "B1": "qke",
                        "B2": "qkep", "B3": "qkea"}[phase]
                for h in range(HPC):
                    _fill(t["K"][h], S)
                    _fill(t["Q"][h], S)
                for i in range(NKC):
                    _fill(t["V"][i], HPC * D)
                with _loopctx():
                    for qp in range(2):
                        _emit_B_half(nc, tc, d, t, p, att_t, qp, mode)
            elif phase == "C":
                for h in range(HPC):
                    _fill(att_t[h], S)
                with _loopctx():
                    for qp in range(2):
                        _emit_C_half(nc, tc, d, t, p, att_t, qp)
            elif phase == "full":
                if n <= 1:
                    _emit_body(nc, tc, d, t, p, att_t)
                else:
                    assert n % 2 == 0
                    with tc.For_i(0, n // 2, 1):
                        _emit_body(nc, tc, d, t, p, att_t)
                        _emit_body(nc, tc, d, t, p, att_t)
            elif phase == "full2":
                # two unrolled iterations (steady-state sim estimation)
                _emit_body(nc, tc, d, t, p, att_t)
                _emit_body(nc, tc, d, t, p, att_t)
            else:
                raise ValueError(phase)

    nc.compile()
    return nc


def _rope_tables():
    inv_freq = 1.0 / (10000.0 ** (np.arange(0, R, 2, dtype=np.float64) / R))
    t = np.arange(S, dtype=np.float64)
    freqs = np.outer(t, inv_freq)                       # (S, R/2)
    emb = np.concatenate([freqs, freqs], axis=-1)       # (S, R)
    cos = np.cos(emb).astype(np.float32)                # (S, R)
    sin = np.sin(emb).astype(np.float32)
    perm = np.array([(j // 2) if j % 2 == 0 else (j // 2) + R // 2
                     for j in range(R)])
    sign = np.array([-1.0 if j % 2 == 0 else 1.0
                     for j in range(R)], dtype=np.float32)
    cos_p = cos[:, perm].T.copy()                       # (R, S)
    sin_p = (sin[:, perm] * sign[None, :]).T.copy()     # (R, S)
    cosT = np.concatenate([cos_p, cos_p], axis=0)       # (128, S)
    sinT = np.concatenate([sin_p, sin_p], axis=0)
    return cosT, sinT, perm


def _bf16():
    import concourse.mybir as mybir
    return mybir.dt.np(mybir.dt.bfloat16)


def _per_core_inputs(inputs, core):
    b, hg = divmod(core, HPC)
    cosT, sinT, perm = _rope_tables()
    hsl64 = np.concatenate([hg * HPC * R + h * R + perm
                            for h in range(HPC)])       # permuted rope rows
    hs64 = slice(hg * HPC * R, (hg + 1) * HPC * R)      # natural 64-rows
    hs128 = slice(hg * HPC * D, (hg + 1) * HPC * D)     # natural 128-rows

    x = np.asarray(inputs["x"], dtype=np.float32)
    f = np.float32
    bf = _bf16()

    def c(a, dt=None):
        return np.ascontiguousarray(a).astype(dt if dt is not None else bf)

    g = {k: np.asarray(v, f) for k, v in inputs.items()}
    # biases: the latent down-projection biases fold into the up biases
    bk1 = g["bk_u"][hs64] + g["Wk_u"][hs64] @ g["bkv_d"]
    bv = g["bv_u"][hs128] + g["Wv_u"][hs128] @ g["bkv_d"]
    bq1 = g["bq_u"][hs64] + g["Wq_u"][hs64] @ g["bq_d"]
    bqr = (g["brq"] + g["Wrq"] @ g["bq_d"])[hsl64]

    lat = _LAT_CACHE.get(b)
    if lat is None:
        lat = np.zeros((2 * LAT, S), dtype=_bf16())

    im = {
        "xT": c(x[b].T),
        "latT": np.ascontiguousarray(lat).astype(_bf16()),
        "wk1u": c(g["Wk_u"][hs64].T),             # (LAT, 256)
        "wq1u": c(g["Wq_u"][hs64].T),
        "wqru": c(g["Wrq"][hsl64].T),             # (LAT, 256), rope-perm
        "wrkT": c(g["Wrk"][hsl64].T),
        "wvu": c(g["Wv_u"][hs128].T),             # (LAT, 512)
        "woT": c(g["Wo"].T[hs128]),
        "bk1f": c(bk1.reshape(2, 128).T, f),
        "bq1f": c(bq1.reshape(2, 128).T, f),
        "bqrf": c(bqr.reshape(2, 128).T, f),
        "brk": c(g["brk"][hsl64].reshape(2, 128).T, f),
        "bvf": c(bv.reshape(1, HPC * D), f),
        "onesd": np.ones((128, 1), dtype=bf),
        "cosT": cosT.astype(bf),
        "sinT": sinT.astype(bf),
    }
    return im


def _per_core_inputs_down(inputs, core):
    """Down-program inputs: x (its batch) + the core's latent shard
    weights (128 rows of Wkv_d and of Wq_d, by core index in the batch
    group)."""
    b, gsh = divmod(core, HPC)
    f = np.float32
    bf = _bf16()
    x = np.asarray(inputs["x"], dtype=f)
    wkv = np.asarray(inputs["Wkv_d"], dtype=f)[gsh * 128:(gsh + 1) * 128]
    wq = np.asarray(inputs["Wq_d"], dtype=f)[gsh * 128:(gsh + 1) * 128]
    wdn = np.concatenate([wkv.T, wq.T], axis=1)   # (E, 256)
    return {
        "xT": np.ascontiguousarray(x[b].T).astype(bf),
        "wdn": np.ascontiguousarray(wdn).astype(bf),
    }


def _gather_latents(down_res):
    """Concatenate the 4 per-core shards of each batch into full
    kv_d/q_d latents; fills _LAT_CACHE."""
    for b in range(B):
        kv = np.concatenate(
            [down_res[b * HPC + gsh]["lat"][0:128] for gsh in range(HPC)],
            axis=0)
        q = np.concatenate(
            [down_res[b * HPC + gsh]["lat"][128:256] for gsh in range(HPC)],
            axis=0)
        _LAT_CACHE[b] = np.concatenate([kv, q], axis=0)  # (2*LAT, S)


def _get_runtime(loop=None, donate=True):
    key = (loop, donate)
    if key in _RT:
        return _RT[key]
    import jax
    import numpy as _np
    from jax.sharding import Mesh, PartitionSpec
    from jax.experimental.shard_map import shard_map

    import concourse.mybir as mybir
    from concourse import bass2jax

    if loop is not None and loop[0] == "down":
        nc = _build_down(loop[1])
    else:
        nc = _build_program(loop)
    bass2jax.install_neuronx_cc_hook()

    partition_name = (nc.partition_id_tensor.name
                      if nc.partition_id_tensor else None)
    in_names, out_names, out_avals, zero_shapes = [], [], [], []
    for alloc in nc.m.functions[0].allocations:
        if not isinstance(alloc, mybir.MemoryLocationSet):
            continue
        name = alloc.memorylocations[0].name
        if alloc.kind == "ExternalInput":
            if name != partition_name:
                in_names.append(name)
        elif alloc.kind == "ExternalOutput":
            out_names.append(name)
            np_dt = mybir.dt.np(alloc.dtype)
            out_avals.append(jax.core.ShapedArray(
                tuple(alloc.tensor_shape), np_dt))
            zero_shapes.append((tuple(alloc.tensor_shape), np_dt))

    n_params = len(in_names)
    n_outs = len(out_names)
    all_in_names = list(in_names) + list(out_names)
    if partition_name is not None:
        all_in_names.append(partition_name)

    def _body(*args):
        operands = list(args)
        if partition_name is not None:
            operands.append(bass2jax.partition_id_tensor())
        outs = bass2jax._bass_exec_p.bind(
            *operands,
            out_avals=tuple(out_avals),
            in_names=tuple(all_in_names),
            out_names=tuple(out_names),
            lowering_input_output_aliases=(),
            sim_require_finite=True,
            sim_require_nnan=True,
            nc=nc,
        )
        return tuple(outs)

    devices = jax.devices()[:NCORES]
    mesh = Mesh(_np.asarray(devices), ("core",))
    in_specs = (PartitionSpec("core"),) * (n_params + n_outs)
    out_specs = (PartitionSpec("core"),) * n_outs
    donate_idx = (tuple(range(n_params, n_params + n_outs)) if donate
                  else ())
    sharded = jax.jit(
        shard_map(_body, mesh=mesh, in_specs=in_specs, out_specs=out_specs,
                  check_rep=False),
        donate_argnums=donate_idx, keep_unused=True)

    _RT[key] = dict(sharded=sharded, in_names=in_names, out_names=out_names,
                    zero_shapes=zero_shapes, n_outs=n_outs)
    return _RT[key]


def _run_cores(in_maps, loop=None):
    rt = _get_runtime(loop)
    import numpy as _np
    concat_in = [
        _np.concatenate([in_maps[c][name] for c in range(NCORES)], axis=0)
        for name in rt["in_names"]
    ]
    concat_zeros = [
        _np.zeros((NCORES * shp[0],) + shp[1:], dt)
        for (shp, dt) in rt["zero_shapes"]
    ]
    out_arrs = rt["sharded"](*concat_in, *concat_zeros)
    res = []
    for c in range(NCORES):
        m = {}
        for i, name in enumerate(rt["out_names"]):
            shp, dt = rt["zero_shapes"][i]
            m[name] = _np.asarray(out_arrs[i]).reshape((NCORES,) + shp)[c]
        res.append(m)
    return res


def kernel(**inputs):
    # program 1: tensor-parallel latent down-projections; host gathers
    # the shards into full latents for each batch group
    down_maps = [_per_core_inputs_down(inputs, c) for c in range(NCORES)]
    down_res = _run_cores(down_maps, loop=("down", 1))
    _gather_latents(down_res)

    # program 2: up-projections from the latents, attention, out-proj
    in_maps = [_per_core_inputs(inputs, c) for c in range(NCORES)]
    res = _run_cores(in_maps)
    bo = np.asarray(inputs["bo"], dtype=np.float32)
    final = np.empty((B, S, E), dtype=np.float32)
    for b in range(B):
        acc = res[HPC * b]["out"].astype(np.float32)
        for g in range(1, HPC):
            acc = acc + res[HPC * b + g]["out"].astype(np.float32)
        final[b] = acc + bo[None, :]
    return final
